# revision 20
# baseline (speedup 1.0000x reference)
"""Trainium2 Bass kernel for TimeSformer-style divided space attention.

Problem: x[4,3137,1024] -> qkv proj (16 heads, dh=64) -> per-frame spatial
attention (cls token attends globally; each frame's 196 patches attend to
frame + cls) -> out proj.

Sharding: 8 cores = 4 batches x 2 head-groups (8 heads each). Each core
computes a full [3137,1024] partial output (its head-group's contribution
through w_out); host sums the two partials per batch.

Transpose-free attention: sim is computed directly in [keys, queries]
orientation (stationary kT, moving qT), with the cls query replicated as
column 196 of every frame block so cls attention rides along the per-frame
matmuls. v carries a ones column so the softmax denominator falls out of
the same PE matmul that produces the output (even heads; odd heads are
forced to PE column-position 64 and use a separate M=1 denominator
matmul). Normalization = DVE reciprocal + gpsimd partition broadcast +
DVE multiply, folded per (head, frame). The cls query over-counts the
cls key 16x (once per frame block); a correction using e0=exp(q_cls.k_cls)
fixes numerator and denominator at the end.

Device layouts (matmul operands bf16, accumulation fp32):
  xt    [1024, 3137]   x[b]^T, host-pretransposed
  wqkv  [1024, 1536]   [q|k|v] column slice for the head group, q pre-scaled
  wout  [512, 1024]    row slice of w_out for the head group
  qTx/kTx sbuf [128, 4, 16*197(+pad)]: partition = (h%2)*64 + d,
        free = (head-pair, frame-block of [196 tokens | cls])
  v_fr  sbuf [128, 32, 8, 65]: frame-aligned token chunks (128 + 68 rows,
        cls_v in partition 68 of odd chunks), free = (head, [v(64) | 1])
  attnT sbuf [128, 4, 3137]: d-major attention output; free col j<3136 is
        token j+1, col 3136 is the cls token
"""

import numpy as np
import ml_dtypes

B = 4
T = 3137          # 1 + 16*196
TP = T - 1        # 3136 patch tokens
D = 1024
NH = 8            # heads per core
DH = 64
F = 16
NP = 196
KB = NP + 1       # 197: per-frame block = 196 patches + cls
HD = NH * DH      # 512
QKV = 3 * HD      # 1536
N_CORES = 8
SCALE = DH ** -0.5
KPAD = 59         # zero pad after kTx blocks so chunk-b sims read defined data

bf16 = ml_dtypes.bfloat16

_CACHE = {}


def _build_nc():
    from concourse import bacc, mybir, tile
    from contextlib import ExitStack

    dt = mybir.dt
    AF = mybir.ActivationFunctionType

    nc = bacc.Bacc(None, target_bir_lowering=False, debug=False)

    xt_d = nc.dram_tensor("xt", [D, T], dt.bfloat16, kind="ExternalInput")
    wqkv_d = nc.dram_tensor("wqkv", [D, QKV], dt.bfloat16, kind="ExternalInput")
    wout_d = nc.dram_tensor("wout", [HD, D], dt.bfloat16, kind="ExternalInput")
    out_d = nc.dram_tensor("out", [T, D], dt.float32, kind="ExternalOutput")

    KD = D // 128  # 8 contraction chunks for the projections

    with tile.TileContext(nc) as tc, ExitStack() as ctx:
        # ---- static tiles (live for the whole kernel) ----
        stat = ctx.enter_context(tc.tile_pool(name="stat", bufs=1))
        wq_sb = stat.tile([128, KD, QKV], dt.bfloat16)
        wout_sb = stat.tile([128, 4, D], dt.bfloat16)
        qTx = stat.tile([128, 4, F * KB], dt.bfloat16)
        kTx = stat.tile([128, 4, F * KB + KPAD], dt.bfloat16)
        clsqk = stat.tile([128, 8], dt.float32)
        v_fr = stat.tile([128, 2 * F, NH, DH + 1], dt.bfloat16)
        vcls = stat.tile([1, NH, DH], dt.bfloat16)
        vclsT = stat.tile([128, 8], dt.bfloat16)
        attnT = stat.tile([128, 4, T], dt.bfloat16)
        cls_st = stat.tile([128, 8, F], dt.float32)
        cls_stden = stat.tile([1, 8, F], dt.float32)
        e0_t = stat.tile([1, 8], dt.bfloat16)
        ones16 = stat.tile([128, 16], dt.float32)

        for k in range(KD):
            nc.sync.dma_start(wq_sb[:, k, :], wqkv_d[k * 128:(k + 1) * 128, :])

        nc.vector.memset(ones16[:, :], 1.0)
        nc.vector.memset(vclsT[:, :], 0.0)
        nc.vector.memset(kTx[:, :, F * KB:], 0.0)
        nc.vector.memset(v_fr[:, :, :, DH:DH + 1], 1.0)
        nc.vector.memset(cls_st[:, :, :], 0.0)
        nc.vector.memset(cls_stden[:, :, :], 0.0)

        # ================= Phase 1+2: projections and attention =================
        # Order: v first, then qk pair 0, then attention of pair c interleaved
        # with qk projection of pair c+1 (PE stays dense, HAM stays warm).
        # out_proj shares its psum slots with the qk chunks (disjoint in time).
        with ExitStack() as p1:
            xt_pool = p1.enter_context(tc.tile_pool(name="xt", bufs=1))
            xt_sb = xt_pool.tile([128, KD, T], dt.bfloat16)
            # token-major slabs so the first v matmul starts early
            tbs = list(range(0, T, 512))
            for t0 in tbs:
                tn = min(512, T - t0)
                for k in range(KD):
                    nc.sync.dma_start(xt_sb[:, k, t0:t0 + tn],
                                      xt_d[k * 128:(k + 1) * 128, t0:t0 + tn])
            for c in range(4):
                nc.sync.dma_start(wout_sb[:, c, :], wout_d[c * 128:(c + 1) * 128, :])

            # ---- v projection (frame-aligned, strided into [v|1] layout) ----
            with ExitStack() as pv:
                ps_v = pv.enter_context(
                    tc.tile_pool(name="ps_v", bufs=2, space="PSUM"))
                for f in range(F):
                    for jc in range(2):
                        r0 = 1 + NP * f + 128 * jc
                        rn = 128 if jc == 0 else NP - 128
                        ps = ps_v.tile([128, NH, DH], dt.float32, tag="v")
                        for k in range(KD):
                            nc.tensor.matmul(
                                ps[:rn, :, :],
                                xt_sb[:, k, r0:r0 + rn],
                                wq_sb[:, k, 2 * HD:3 * HD],
                                start=(k == 0), stop=(k == KD - 1))
                        nc.vector.tensor_copy(
                            v_fr[:rn, 2 * f + jc, :, 0:DH], ps[:rn, :, :])
                # cls token's v row
                ps = ps_v.tile([128, NH, DH], dt.float32, tag="v")
                for k in range(KD):
                    nc.tensor.matmul(
                        ps[:1, :, :], xt_sb[:, k, 0:1],
                        wq_sb[:, k, 2 * HD:3 * HD],
                        start=(k == 0), stop=(k == KD - 1))
                nc.vector.tensor_copy(vcls[:, :, :], ps[:1, :, :])
            # scatter cls_v into partition 68 of every odd v chunk
            for f in range(F):
                nc.sync.dma_start(v_fr[68:69, 2 * f + 1, :, 0:DH], vcls[:, :, :])
            # vclsT[d, h] for the cls correction (even heads rows 0:64,
            # odd heads rows 64:128, matching cls_st row layout)
            for h in range(NH):
                if h % 2 == 0:
                    nc.sync.dma_start(vclsT[0:DH, h:h + 1], vcls[:, h, :])
                else:
                    nc.sync.dma_start(vclsT[DH:2 * DH, h:h + 1], vcls[:, h, :])

            ps_gen = p1.enter_context(
                tc.tile_pool(name="ps_gen", bufs=2, space="PSUM"))
            ps_sim = p1.enter_context(
                tc.tile_pool(name="ps_sim", bufs=2, space="PSUM"))
            ps_ot = p1.enter_context(
                tc.tile_pool(name="ps_ot", bufs=2, space="PSUM"))
            ps_den = p1.enter_context(
                tc.tile_pool(name="ps_den", bufs=2, space="PSUM"))
            sb_p = p1.enter_context(tc.tile_pool(name="sb_p", bufs=3))
            sb_r = p1.enter_context(tc.tile_pool(name="sb_r", bufs=2))
            sb_o = p1.enter_context(tc.tile_pool(name="sb_o", bufs=2))

            def qk_chunk(m, c):
                dst = qTx if m < 4 else kTx
                mc = m % 4
                t0 = 0 if c == 0 else 392 * c + 1
                nsz = 393 if c == 0 else 392
                ps = ps_gen.tile([128, 512], dt.float32, tag="g")
                for k in range(KD):
                    nc.tensor.matmul(
                        ps[:, :nsz],
                        wq_sb[:, k, m * 128:(m + 1) * 128],
                        xt_sb[:, k, t0:t0 + nsz],
                        start=(k == 0), stop=(k == KD - 1))
                s0 = 1 if c == 0 else 0
                if c == 0:
                    nc.vector.tensor_copy(clsqk[:, m:m + 1], ps[:, 0:1])
                if m < 4:
                    nc.scalar.copy(dst[:, mc, KB * 2 * c:KB * 2 * c + NP],
                                   ps[:, s0:s0 + NP])
                    nc.scalar.copy(
                        dst[:, mc, KB * (2 * c + 1):KB * (2 * c + 1) + NP],
                        ps[:, s0 + NP:s0 + 2 * NP])
                else:
                    nc.vector.tensor_copy(
                        dst[:, mc, KB * 2 * c:KB * 2 * c + NP],
                        ps[:, s0:s0 + NP])
                    nc.vector.tensor_copy(
                        dst[:, mc, KB * (2 * c + 1):KB * (2 * c + 1) + NP],
                        ps[:, s0 + NP:s0 + 2 * NP])

            def qk_replicate(m):
                # cls q/k into every frame block's column 196
                dst = qTx if m < 4 else kTx
                nc.vector.tensor_scalar_mul(
                    dst[:, m % 4, NP:NP + (F - 1) * KB + 1:KB],
                    ones16[:, 0:F], clsqk[:, m:m + 1])

            def out_proj(c):
                """project attnT cols [128c, 128c+tn) through w_out"""
                t0 = 128 * c
                tn = min(128, T - t0)
                o_sb = sb_o.tile([128, D], dt.float32, tag="o")
                for half in range(2):
                    ps = ps_gen.tile([128, 512], dt.float32, tag="g")
                    for kc in range(4):
                        nc.tensor.matmul(
                            ps[:tn, :],
                            attnT[:, kc, t0:t0 + tn],
                            wout_sb[:, kc, half * 512:(half + 1) * 512],
                            start=(kc == 0), stop=(kc == 3))
                    if half == 0:
                        nc.scalar.copy(o_sb[:tn, 0:512], ps[:tn, :])
                    else:
                        nc.vector.tensor_copy(o_sb[:tn, 512:1024], ps[:tn, :])
                if tn == 128:
                    nc.sync.dma_start(out_d[1 + t0:1 + t0 + tn, :], o_sb[:tn, :])
                else:
                    # final chunk: 64 patch rows + the cls row
                    nc.sync.dma_start(out_d[1 + t0:T, :], o_sb[:tn - 1, :])
                    nc.sync.dma_start(out_d[0:1, :], o_sb[tn - 1:tn, :])

            def attn_hf(h, f):
                hc, pb = h // 2, 64 * (h % 2)
                even = (h % 2 == 0)
                fr0 = KB * f
                sim = ps_sim.tile([128, 2, KB], dt.float32, tag="sim")
                nc.tensor.matmul(
                    sim[:, 0, :],
                    kTx[pb:pb + 64, hc, fr0:fr0 + 128],
                    qTx[pb:pb + 64, hc, fr0:fr0 + KB],
                    start=True, stop=True)
                nc.tensor.matmul(
                    sim[:, 1, :],
                    kTx[pb:pb + 64, hc, fr0 + 128:fr0 + 256],
                    qTx[pb:pb + 64, hc, fr0:fr0 + KB],
                    start=True, stop=True)
                pT = sb_p.tile([128, 2, KB], dt.bfloat16, tag="p")
                nc.scalar.activation(pT[:, :, :], sim[:, :, :], AF.Exp)
                if f == 0:
                    # e0 = exp(q_cls.k_cls) sits at partition 68; engines need
                    # 32-aligned partition bases, DMA does not
                    nc.sync.dma_start(
                        e0_t[0:1, h:h + 1], pT[68:69, 1, NP:NP + 1])
                # attention output (d-major, 64 rows at dlo) + denominator via
                # the v ones-column into a separate bank (partition 0: HW
                # partition_broadcast only works input@0 -> output from 0)
                dlo = 0 if even else 64
                ot = ps_ot.tile([128, 2, KB], dt.float32, tag="ot")
                den = ps_den.tile([128, KB], dt.float32, tag="den")
                nc.tensor.matmul(
                    ot[dlo:dlo + 64, 0, :],
                    v_fr[:, 2 * f, h, 0:DH], pT[:, 0, :],
                    start=True, stop=False)
                nc.tensor.matmul(
                    ot[dlo:dlo + 64, 0, :],
                    v_fr[0:69, 2 * f + 1, h, 0:DH], pT[0:69, 1, :],
                    start=False, stop=True)
                nc.tensor.matmul(
                    den[0:1, :],
                    v_fr[:, 2 * f, h, DH:DH + 1], pT[:, 0, :],
                    start=True, stop=False)
                nc.tensor.matmul(
                    den[0:1, :],
                    v_fr[0:69, 2 * f + 1, h, DH:DH + 1], pT[0:69, 1, :],
                    start=False, stop=True)
                r_t = sb_r.tile([128, NP], dt.float32, tag="r")
                rb = sb_r.tile([128, NP], dt.float32, tag="rb")
                nc.vector.reciprocal_approx_fast(r_t[0:1, :], den[0:1, 0:NP])
                nc.gpsimd.partition_broadcast(rb[:, :], r_t[0:1, :])
                nc.vector.tensor_mul(
                    attnT[dlo:dlo + 64, hc, NP * f:NP * (f + 1)],
                    ot[dlo:dlo + 64, 0, 0:NP], rb[dlo:dlo + 64, :])
                # cls column: stage unnormalized numerator + den per frame
                nc.scalar.copy(cls_st[dlo:dlo + 64, h, f:f + 1],
                               ot[dlo:dlo + 64, 0, NP:KB])
                nc.vector.tensor_copy(cls_stden[0:1, h, f:f + 1],
                                      den[0:1, NP:KB])

            next_chunk = [0]

            def drain_out_proj(done_cols):
                while (next_chunk[0] + 1) * 128 <= done_cols:
                    out_proj(next_chunk[0])
                    next_chunk[0] += 1

            # qk pair 0
            for m in (0, 4):
                for c in range(8):
                    qk_chunk(m, c)
                qk_replicate(m)
            # attention pair c interleaved with qk projection of pair c+1
            for pair in range(3):
                seq = [(pair + 1, c) for c in range(8)] + \
                      [(pair + 5, c) for c in range(8)]
                for i, (m, c) in enumerate(seq):
                    qk_chunk(m, c)
                    if c == 7:
                        qk_replicate(m)
                    attn_hf(2 * pair, i)
                    attn_hf(2 * pair + 1, i)
            # last pair: attention + out-proj of completed token chunks
            for f in range(F):
                attn_hf(6, f)
                attn_hf(7, f)
                drain_out_proj(NP * (f + 1))

            # ---- cls finalization ----
            acc = sb_r.tile([128, 8], dt.float32, tag="acc")
            accden = sb_r.tile([128, 8], dt.float32, tag="accden")
            e0b = sb_r.tile([128, 8], dt.bfloat16, tag="e0b")
            corr = sb_r.tile([128, 8], dt.float32, tag="corr")
            rc = sb_r.tile([128, 8], dt.float32, tag="rc")
            rcb = sb_r.tile([128, 8], dt.float32, tag="rcb")
            nc.vector.reduce_sum(acc[:, :], cls_st[:, :, :],
                                 axis=mybir.AxisListType.X)
            nc.vector.reduce_sum(accden[0:1, :], cls_stden[0:1, :, :],
                                 axis=mybir.AxisListType.X)
            nc.gpsimd.partition_broadcast(e0b[:, :], e0_t[0:1, :])
            # corr = 15*e0*vclsT ; num = acc - corr
            nc.vector.scalar_tensor_tensor(
                corr[:, :], e0b[:, :], float(F - 1), vclsT[:, :],
                mybir.AluOpType.mult, mybir.AluOpType.mult)
            nc.vector.tensor_sub(acc[:, :], acc[:, :], corr[:, :])
            # denominators (all heads) live at partition 0: den -= 15*e0
            nc.vector.tensor_scalar_mul(corr[0:1, :], e0b[0:1, :], float(F - 1))
            nc.vector.tensor_sub(accden[0:1, :], accden[0:1, :], corr[0:1, :])
            nc.vector.reciprocal_approx_fast(rc[0:1, :], accden[0:1, :])
            nc.gpsimd.partition_broadcast(rcb[:, :], rc[0:1, :])
            for h in range(NH):
                hc = h // 2
                if h % 2 == 0:
                    nc.vector.tensor_mul(
                        attnT[0:64, hc, T - 1:T], acc[0:64, h:h + 1],
                        rcb[0:64, h:h + 1])
                else:
                    nc.vector.tensor_mul(
                        attnT[64:128, hc, T - 1:T], acc[64:128, h:h + 1],
                        rcb[64:128, h:h + 1])

            # final out-proj chunk (last 64 patch tokens + cls)
            drain_out_proj(TP)
            out_proj(TP // 128)

    nc.compile()
    return nc


def _get_nc():
    if "nc" not in _CACHE:
        _CACHE["nc"] = _build_nc()
    return _CACHE["nc"]


def _prep_in_maps(x, w_qkv, w_out):
    x = np.asarray(x, dtype=np.float32)
    w_qkv = np.asarray(w_qkv, dtype=np.float32)
    w_out = np.asarray(w_out, dtype=np.float32)
    in_maps = []
    for core in range(N_CORES):
        b, hg = divmod(core, 2)
        xt = np.ascontiguousarray(x[b].T).astype(bf16)
        cs = slice(hg * HD, (hg + 1) * HD)
        wq = np.concatenate(
            [w_qkv[:, 0 * D:][:, cs] * SCALE, w_qkv[:, 1 * D:][:, cs],
             w_qkv[:, 2 * D:][:, cs]], axis=1).astype(bf16)
        wo = np.ascontiguousarray(w_out[hg * HD:(hg + 1) * HD, :]).astype(bf16)
        in_maps.append({"xt": xt, "wqkv": wq, "wout": wo})
    return in_maps


def run(x, w_qkv, w_out, trace=False):
    from concourse.bass_utils import run_bass_kernel_spmd

    nc = _get_nc()
    in_maps = _prep_in_maps(x, w_qkv, w_out)
    res = run_bass_kernel_spmd(nc, in_maps, list(range(N_CORES)), trace=trace)
    out = np.empty((B, T, D), dtype=np.float32)
    for b in range(B):
        out[b] = res.results[2 * b]["out"] + res.results[2 * b + 1]["out"]
    return out, res


def kernel(x, w_qkv, w_out, f):
    assert int(f) == F
    out, _ = run(x, w_qkv, w_out, trace=False)
    return out


# revision 23
# speedup vs baseline: 1.2073x; 1.2073x over previous
"""Trainium2 Bass kernel for TimeSformer-style divided space attention.

Problem: x[4,3137,1024] -> qkv proj (16 heads, dh=64) -> per-frame spatial
attention (cls token attends globally; each frame's 196 patches attend to
frame + cls) -> out proj.

Sharding: 8 cores = 4 batches x 2 head-groups (8 heads each). Each core
computes a full [3137,1024] partial output (its head-group's contribution
through w_out); host sums the two partials per batch.

Transpose-free attention: sim is computed directly in [keys, queries]
orientation (stationary kT, moving qT), with the cls query replicated as
column 196 of every frame block so cls attention rides along the per-frame
matmuls. v carries a ones column so the softmax denominator falls out of
the same PE matmul that produces the output (even heads; odd heads are
forced to PE column-position 64 and use a separate M=1 denominator
matmul). Normalization = DVE reciprocal + gpsimd partition broadcast +
DVE multiply, folded per (head, frame). The cls query over-counts the
cls key 16x (once per frame block); a correction using e0=exp(q_cls.k_cls)
fixes numerator and denominator at the end.

Device layouts (matmul operands bf16, accumulation fp32):
  xt    [1024, 3137]   x[b]^T, host-pretransposed
  wqkv  [1024, 1536]   [q|k|v] column slice for the head group, q pre-scaled
  wout  [512, 1024]    row slice of w_out for the head group
  qTx/kTx sbuf [128, 4, 16*197(+pad)]: partition = (h%2)*64 + d,
        free = (head-pair, frame-block of [196 tokens | cls])
  v_fr  sbuf [128, 32, 8, 65]: frame-aligned token chunks (128 + 68 rows,
        cls_v in partition 68 of odd chunks), free = (head, [v(64) | 1])
  attnT sbuf [128, 4, 3137]: d-major attention output; free col j<3136 is
        token j+1, col 3136 is the cls token
"""

import numpy as np
import ml_dtypes

B = 4
T = 3137          # 1 + 16*196
TP = T - 1        # 3136 patch tokens
D = 1024
NH = 8            # heads per core
DH = 64
F = 16
NP = 196
KB = NP + 1       # 197: per-frame block = 196 patches + cls
HD = NH * DH      # 512
QKV = 3 * HD      # 1536
N_CORES = 8
SCALE = DH ** -0.5
KPAD = 59         # zero pad after kTx blocks so chunk-b sims read defined data

bf16 = ml_dtypes.bfloat16

_CACHE = {}


def _build_nc():
    from concourse import bacc, mybir, tile
    from contextlib import ExitStack

    dt = mybir.dt
    AF = mybir.ActivationFunctionType

    nc = bacc.Bacc(None, target_bir_lowering=False, debug=False)

    xt_d = nc.dram_tensor("xt", [D, T], dt.bfloat16, kind="ExternalInput")
    wqkv_d = nc.dram_tensor("wqkv", [D, QKV], dt.bfloat16, kind="ExternalInput")
    wout_d = nc.dram_tensor("wout", [HD, D], dt.bfloat16, kind="ExternalInput")
    out_d = nc.dram_tensor("out", [T, D], dt.float32, kind="ExternalOutput")

    KD = D // 128  # 8 contraction chunks for the projections

    with tile.TileContext(nc) as tc, ExitStack() as ctx:
        # ---- static tiles (live for the whole kernel) ----
        stat = ctx.enter_context(tc.tile_pool(name="stat", bufs=1))
        wq_sb = stat.tile([128, KD, QKV], dt.bfloat16)
        wout_sb = stat.tile([128, 4, D], dt.bfloat16)
        qTx = stat.tile([128, 4, F * KB], dt.bfloat16)
        kTx = stat.tile([128, 4, F * KB + KPAD], dt.bfloat16)
        clsqk = stat.tile([128, 8], dt.float32)
        v_fr = stat.tile([128, 2 * F, NH, DH + 1], dt.bfloat16)
        vcls = stat.tile([1, NH, DH], dt.bfloat16)
        vclsT = stat.tile([128, 8], dt.bfloat16)
        attnT = stat.tile([128, 4, T], dt.bfloat16)
        cls_st = stat.tile([128, 8, F], dt.float32)
        cls_stden = stat.tile([1, 8, F], dt.float32)
        e0_t = stat.tile([1, 8], dt.bfloat16)
        ones16 = stat.tile([128, 16], dt.float32)

        for k in range(KD):
            nc.sync.dma_start(wq_sb[:, k, 2 * HD:3 * HD],
                                wqkv_d[k * 128:(k + 1) * 128, 2 * HD:3 * HD])

        nc.vector.memset(ones16[:, :], 1.0)
        nc.vector.memset(vclsT[:, :], 0.0)
        nc.vector.memset(kTx[:, :, F * KB:], 0.0)
        nc.vector.memset(v_fr[:, :, :, DH:DH + 1], 1.0)
        nc.vector.memset(cls_st[:, :, :], 0.0)
        nc.vector.memset(cls_stden[:, :, :], 0.0)

        # ================= Phase 1+2: projections and attention =================
        # Order: v first, then qk pair 0, then attention of pair c interleaved
        # with qk projection of pair c+1 (PE stays dense, HAM stays warm).
        # out_proj shares its psum slots with the qk chunks (disjoint in time).
        with ExitStack() as p1:
            xt_pool = p1.enter_context(tc.tile_pool(name="xt", bufs=1))
            xt_sb = xt_pool.tile([128, KD, T], dt.bfloat16)
            # token splits so the first v matmuls start early
            for t0, t1 in ((0, 1056), (1056, 2112), (2112, T)):
                for k in range(KD):
                    nc.sync.dma_start(xt_sb[:, k, t0:t1],
                                        xt_d[k * 128:(k + 1) * 128, t0:t1])
            for k in range(KD):
                nc.sync.dma_start(wq_sb[:, k, 0:2 * HD],
                                    wqkv_d[k * 128:(k + 1) * 128, 0:2 * HD])
            for c in range(4):
                nc.sync.dma_start(wout_sb[:, c, :],
                                    wout_d[c * 128:(c + 1) * 128, :])

            # ---- v projection (frame-aligned, strided into [v|1] layout) ----
            with ExitStack() as pv:
                ps_v = pv.enter_context(
                    tc.tile_pool(name="ps_v", bufs=2, space="PSUM"))
                for f in range(F):
                    for jc in range(2):
                        r0 = 1 + NP * f + 128 * jc
                        rn = 128 if jc == 0 else NP - 128
                        ps = ps_v.tile([128, NH, DH], dt.float32, tag="v")
                        for k in range(KD):
                            nc.tensor.matmul(
                                ps[:rn, :, :],
                                xt_sb[:, k, r0:r0 + rn],
                                wq_sb[:, k, 2 * HD:3 * HD],
                                start=(k == 0), stop=(k == KD - 1))
                        nc.vector.tensor_copy(
                            v_fr[:rn, 2 * f + jc, :, 0:DH], ps[:rn, :, :])
                # cls token's v row
                ps = ps_v.tile([128, NH, DH], dt.float32, tag="v")
                for k in range(KD):
                    nc.tensor.matmul(
                        ps[:1, :, :], xt_sb[:, k, 0:1],
                        wq_sb[:, k, 2 * HD:3 * HD],
                        start=(k == 0), stop=(k == KD - 1))
                nc.vector.tensor_copy(vcls[:, :, :], ps[:1, :, :])
            # scatter cls_v into partition 68 of every odd v chunk
            for f in range(F):
                nc.sync.dma_start(v_fr[68:69, 2 * f + 1, :, 0:DH], vcls[:, :, :])
            # vclsT[d, h] for the cls correction (even heads rows 0:64,
            # odd heads rows 64:128, matching cls_st row layout)
            for h in range(NH):
                if h % 2 == 0:
                    nc.sync.dma_start(vclsT[0:DH, h:h + 1], vcls[:, h, :])
                else:
                    nc.sync.dma_start(vclsT[DH:2 * DH, h:h + 1], vcls[:, h, :])

            ps_gen = p1.enter_context(
                tc.tile_pool(name="ps_gen", bufs=2, space="PSUM"))
            ps_sim = p1.enter_context(
                tc.tile_pool(name="ps_sim", bufs=3, space="PSUM"))
            ps_ot = p1.enter_context(
                tc.tile_pool(name="ps_ot", bufs=3, space="PSUM"))
            sb_p = p1.enter_context(tc.tile_pool(name="sb_p", bufs=4))
            sb_r = p1.enter_context(tc.tile_pool(name="sb_r", bufs=2))
            sb_o = p1.enter_context(tc.tile_pool(name="sb_o", bufs=2))

            def qk_chunk(m, c):
                dst = qTx if m < 4 else kTx
                mc = m % 4
                t0 = 0 if c == 0 else 392 * c + 1
                nsz = 393 if c == 0 else 392
                ps = ps_gen.tile([128, 512], dt.float32, tag="g")
                for k in range(KD):
                    nc.tensor.matmul(
                        ps[:, :nsz],
                        wq_sb[:, k, m * 128:(m + 1) * 128],
                        xt_sb[:, k, t0:t0 + nsz],
                        start=(k == 0), stop=(k == KD - 1))
                s0 = 1 if c == 0 else 0
                if c == 0:
                    nc.vector.tensor_copy(clsqk[:, m:m + 1], ps[:, 0:1])
                if m < 4:
                    nc.scalar.copy(dst[:, mc, KB * 2 * c:KB * 2 * c + NP],
                                   ps[:, s0:s0 + NP])
                    nc.scalar.copy(
                        dst[:, mc, KB * (2 * c + 1):KB * (2 * c + 1) + NP],
                        ps[:, s0 + NP:s0 + 2 * NP])
                else:
                    nc.vector.tensor_copy(
                        dst[:, mc, KB * 2 * c:KB * 2 * c + NP],
                        ps[:, s0:s0 + NP])
                    nc.vector.tensor_copy(
                        dst[:, mc, KB * (2 * c + 1):KB * (2 * c + 1) + NP],
                        ps[:, s0 + NP:s0 + 2 * NP])

            def qk_replicate(m):
                # cls q/k into every frame block's column 196
                dst = qTx if m < 4 else kTx
                nc.vector.tensor_scalar_mul(
                    dst[:, m % 4, NP:NP + (F - 1) * KB + 1:KB],
                    ones16[:, 0:F], clsqk[:, m:m + 1])

            def out_proj(c):
                """project attnT cols [128c, 128c+tn) through w_out"""
                t0 = 128 * c
                tn = min(128, T - t0)
                o_sb = sb_o.tile([128, D], dt.float32, tag="o")
                for half in range(2):
                    ps = ps_gen.tile([128, 512], dt.float32, tag="g")
                    for kc in range(4):
                        nc.tensor.matmul(
                            ps[:tn, :],
                            attnT[:, kc, t0:t0 + tn],
                            wout_sb[:, kc, half * 512:(half + 1) * 512],
                            start=(kc == 0), stop=(kc == 3))
                    if half == 0:
                        nc.scalar.copy(o_sb[:tn, 0:512], ps[:tn, :])
                    else:
                        nc.vector.tensor_copy(o_sb[:tn, 512:1024], ps[:tn, :])
                if tn == 128:
                    nc.sync.dma_start(out_d[1 + t0:1 + t0 + tn, :], o_sb[:tn, :])
                else:
                    # final chunk: 64 patch rows + the cls row
                    nc.sync.dma_start(out_d[1 + t0:T, :], o_sb[:tn - 1, :])
                    nc.sync.dma_start(out_d[0:1, :], o_sb[tn - 1:tn, :])

            def attn_hf(h, f):
                hc, pb = h // 2, 64 * (h % 2)
                even = (h % 2 == 0)
                fr0 = KB * f
                sim = ps_sim.tile([128, 2, KB], dt.float32, tag="sim")
                nc.tensor.matmul(
                    sim[:, 0, :],
                    kTx[pb:pb + 64, hc, fr0:fr0 + 128],
                    qTx[pb:pb + 64, hc, fr0:fr0 + KB],
                    start=True, stop=True)
                nc.tensor.matmul(
                    sim[:, 1, :],
                    kTx[pb:pb + 64, hc, fr0 + 128:fr0 + 256],
                    qTx[pb:pb + 64, hc, fr0:fr0 + KB],
                    start=True, stop=True)
                pT = sb_p.tile([128, 2, KB], dt.bfloat16, tag="p")
                nc.scalar.activation(pT[:, :, :], sim[:, :, :], AF.Exp)
                if f == 0:
                    # e0 = exp(q_cls.k_cls) sits at partition 68; engines need
                    # 32-aligned partition bases, DMA does not
                    nc.sync.dma_start(
                        e0_t[0:1, h:h + 1], pT[68:69, 1, NP:NP + 1])
                # attention output + denominator in one matmul pair via the
                # v ones-column: out rows 0..63 = d, row 64 = sum(exp).
                # DVE ops partition-shift from PSUM where needed (HW-verified)
                dlo = 0 if even else 64
                ot = ps_ot.tile([128, 2, KB], dt.float32, tag="ot")
                nc.tensor.matmul(
                    ot[0:65, 0, :],
                    v_fr[:, 2 * f, h, 0:DH + 1], pT[:, 0, :],
                    start=True, stop=False)
                nc.tensor.matmul(
                    ot[0:65, 0, :],
                    v_fr[0:69, 2 * f + 1, h, 0:DH + 1], pT[0:69, 1, :],
                    start=False, stop=True)
                r_t = sb_r.tile([128, NP], dt.float32, tag="r")
                r2 = sb_r.tile([128, NP], dt.float32, tag="r2")
                rb = sb_r.tile([128, NP], dt.float32, tag="rb")
                # custom-DVE recip can't partition-shift from PSUM: stage via
                # a plain copy (shift-capable), then recip on aligned SBUF
                nc.vector.tensor_copy(r2[0:1, :], ot[64:65, 0, 0:NP])
                nc.vector.reciprocal_approx_fast(r_t[0:1, :], r2[0:1, :])
                nc.gpsimd.partition_broadcast(rb[:, :], r_t[0:1, :])
                nc.vector.tensor_mul(
                    attnT[dlo:dlo + 64, hc, NP * f:NP * (f + 1)],
                    ot[0:64, 0, 0:NP], rb[dlo:dlo + 64, :])
                # cls column: stage unnormalized numerator + den per frame
                nc.scalar.copy(cls_st[dlo:dlo + 64, h, f:f + 1],
                               ot[0:64, 0, NP:KB])
                nc.vector.tensor_copy(cls_stden[0:1, h, f:f + 1],
                                      ot[64:65, 0, NP:KB])

            next_chunk = [0]

            def drain_out_proj(done_cols):
                while (next_chunk[0] + 1) * 128 <= done_cols:
                    out_proj(next_chunk[0])
                    next_chunk[0] += 1

            # qk pair 0
            for m in (0, 4):
                for c in range(8):
                    qk_chunk(m, c)
                qk_replicate(m)
            # attention pair c interleaved with qk projection of pair c+1
            for pair in range(3):
                seq = [(pair + 1, c) for c in range(8)] + \
                      [(pair + 5, c) for c in range(8)]
                for i, (m, c) in enumerate(seq):
                    qk_chunk(m, c)
                    if c == 7:
                        qk_replicate(m)
                    attn_hf(2 * pair, i)
                    attn_hf(2 * pair + 1, i)
            # last pair: attention + out-proj of completed token chunks
            for f in range(F):
                attn_hf(6, f)
                attn_hf(7, f)
                drain_out_proj(NP * (f + 1))

            # ---- cls finalization ----
            acc = sb_r.tile([128, 8], dt.float32, tag="acc")
            accden = sb_r.tile([128, 8], dt.float32, tag="accden")
            e0b = sb_r.tile([128, 8], dt.bfloat16, tag="e0b")
            corr = sb_r.tile([128, 8], dt.float32, tag="corr")
            rc = sb_r.tile([128, 8], dt.float32, tag="rc")
            rcb = sb_r.tile([128, 8], dt.float32, tag="rcb")
            nc.vector.reduce_sum(acc[:, :], cls_st[:, :, :],
                                 axis=mybir.AxisListType.X)
            nc.vector.reduce_sum(accden[0:1, :], cls_stden[0:1, :, :],
                                 axis=mybir.AxisListType.X)
            nc.gpsimd.partition_broadcast(e0b[:, :], e0_t[0:1, :])
            # corr = 15*e0*vclsT ; num = acc - corr
            nc.vector.scalar_tensor_tensor(
                corr[:, :], e0b[:, :], float(F - 1), vclsT[:, :],
                mybir.AluOpType.mult, mybir.AluOpType.mult)
            nc.vector.tensor_sub(acc[:, :], acc[:, :], corr[:, :])
            # denominators (all heads) live at partition 0: den -= 15*e0
            nc.vector.tensor_scalar_mul(corr[0:1, :], e0b[0:1, :], float(F - 1))
            nc.vector.tensor_sub(accden[0:1, :], accden[0:1, :], corr[0:1, :])
            nc.vector.reciprocal_approx_fast(rc[0:1, :], accden[0:1, :])
            nc.gpsimd.partition_broadcast(rcb[:, :], rc[0:1, :])
            for h in range(NH):
                hc = h // 2
                if h % 2 == 0:
                    nc.vector.tensor_mul(
                        attnT[0:64, hc, T - 1:T], acc[0:64, h:h + 1],
                        rcb[0:64, h:h + 1])
                else:
                    nc.vector.tensor_mul(
                        attnT[64:128, hc, T - 1:T], acc[64:128, h:h + 1],
                        rcb[64:128, h:h + 1])

            # final out-proj chunk (last 64 patch tokens + cls)
            drain_out_proj(TP)
            out_proj(TP // 128)

    nc.compile()
    return nc


def _get_nc():
    if "nc" not in _CACHE:
        _CACHE["nc"] = _build_nc()
    return _CACHE["nc"]


def _prep_in_maps(x, w_qkv, w_out):
    x = np.asarray(x, dtype=np.float32)
    w_qkv = np.asarray(w_qkv, dtype=np.float32)
    w_out = np.asarray(w_out, dtype=np.float32)
    in_maps = []
    for core in range(N_CORES):
        b, hg = divmod(core, 2)
        xt = np.ascontiguousarray(x[b].T).astype(bf16)
        cs = slice(hg * HD, (hg + 1) * HD)
        wq = np.concatenate(
            [w_qkv[:, 0 * D:][:, cs] * SCALE, w_qkv[:, 1 * D:][:, cs],
             w_qkv[:, 2 * D:][:, cs]], axis=1).astype(bf16)
        wo = np.ascontiguousarray(w_out[hg * HD:(hg + 1) * HD, :]).astype(bf16)
        in_maps.append({"xt": xt, "wqkv": wq, "wout": wo})
    return in_maps


def run(x, w_qkv, w_out, trace=False):
    from concourse.bass_utils import run_bass_kernel_spmd

    nc = _get_nc()
    in_maps = _prep_in_maps(x, w_qkv, w_out)
    res = run_bass_kernel_spmd(nc, in_maps, list(range(N_CORES)), trace=trace)
    out = np.empty((B, T, D), dtype=np.float32)
    for b in range(B):
        out[b] = res.results[2 * b]["out"] + res.results[2 * b + 1]["out"]
    return out, res


def kernel(x, w_qkv, w_out, f):
    assert int(f) == F
    out, _ = run(x, w_qkv, w_out, trace=False)
    return out
